# revision 41
# baseline (speedup 1.0000x reference)
import sys
sys.path.insert(0, '/opt/trn_rl_repo')
import numpy as np
import concourse.bass as bass
import concourse.bacc as bacc
import concourse.tile as tile
from concourse import mybir
from concourse.bass_utils import run_bass_kernel_spmd

F32 = mybir.dt.float32
F16 = mybir.dt.float16      # half precision for matmul operands
AF = mybir.ActivationFunctionType
ALU = mybir.AluOpType

U = 400        # LSTM units
KATT = 10     # attention gaussians
NCHARS = 73   # alphabet
NMIX = 20     # GMM components
UC = 50       # char positions
NB = 4        # batch per core
NCORES = 8
NOUT = 6 * NMIX + 1  # 121

# m-tiles: per gate [128,128,128,16] -> 16 m-tiles, psum z [128, 64]
MW = [128, 128, 128, 16] * 4
SOFF = [400 * (m // 4) + 128 * (m % 4) for m in range(16)]   # W col offsets
TAILP = 96          # partition base for unit-tail (384:400) values
WROWS = 77          # mix rows: chars (perm), x @64:67, bias @76
OHROWS = 99         # phiaug/OHB rows: phi @0:50, one @64, x @96:99

_CACHE = {}
F16_KEYS = {'W1', 'W2', 'W3', 'XT', 'WMDN', 'HB'}


def _char_row(c):
    # char c's partition in the mix/wp layout (64:67 reserved for x)
    return c if c < 64 else 67 + (c - 64)


def _build_program(T):
    nc = bacc.Bacc("TRN2", target_bir_lowering=False, debug=False, num_devices=NCORES)

    dW1 = nc.dram_tensor("W1", [128, 4 * 1600], F16, kind="ExternalInput").ap()
    dW2 = nc.dram_tensor("W2", [128, 8 * 1600], F16, kind="ExternalInput").ap()
    dW3 = nc.dram_tensor("W3", [128, 8 * 1600], F16, kind="ExternalInput").ap()
    dPB = nc.dram_tensor("PB", [128, 9 * 16], F32, kind="ExternalInput").ap()
    dXT = nc.dram_tensor("XT", [3, T * NB], F16, kind="ExternalInput").ap()
    dWATT = nc.dram_tensor("WATT", [128, 4 * 30], F32, kind="ExternalInput").ap()
    dV3 = nc.dram_tensor("V3", [1, 3 * UC], F32, kind="ExternalInput").ap()
    dOHB = nc.dram_tensor("OHB", [OHROWS, NB * WROWS], F32, kind="ExternalInput").ap()
    dWMDN = nc.dram_tensor("WMDN", [128, 4 * 200], F16, kind="ExternalInput").ap()
    dHB = nc.dram_tensor("HB", [128, 16], F16, kind="ExternalInput").ap()
    dOUT1 = nc.dram_tensor("OUT1", [128, T * NB], F32, kind="ExternalOutput").ap()
    dOUT2 = nc.dram_tensor("OUT2", [72, T * NB], F32, kind="ExternalOutput").ap()

    with tile.TileContext(nc) as tc:
        with tc.tile_pool(name="statics", bufs=1) as statics, \
             tc.tile_pool(name="states", bufs=1) as states:

            sW1 = statics.tile([128, 4 * 1600], F16)
            sW2 = statics.tile([128, 8 * 1600], F16)
            sW3 = statics.tile([128, 8 * 1600], F16)
            sPB = statics.tile([128, 9 * 16], F32)
            sXT = statics.tile([3, T * NB], F16)
            sWATT = statics.tile([128, 4 * 30], F32)
            sV3 = statics.tile([1, 3 * UC], F32)
            sOHB = statics.tile([OHROWS, NB * WROWS], F32)
            sWMDN = statics.tile([128, 4 * 200], F16)
            sHB = statics.tile([128, 16], F16)
            for dst, src in [(sW1, dW1), (sW2, dW2), (sW3, dW3), (sPB, dPB),
                             (sXT, dXT), (sWATT, dWATT), (sV3, dV3),
                             (sOHB, dOHB), (sWMDN, dWMDN), (sHB, dHB)]:
                nc.gpsimd.dma_start(out=dst[:], in_=src[:])

            h3all = states.tile([128, T * 16], F16)
            OUTS1 = states.tile([128, T * NB], F32)
            OUTS2 = states.tile([72, T * NB], F32)
            h = [states.tile([128, 16], F16, name=f"h{i}") for i in range(3)]
            h0f = states.tile([128, 16], F32, name="h0f")   # fp32 h1 for attention
            # c kept duplicated [c|c] so the i/f gate chains fuse into one op
            c = [states.tile([128, 32], F32, name=f"c{i}") for i in range(3)]
            mix = [states.tile([128, NB], F16, name=f"mix{i}") for i in range(3)]
            kap = states.tile([1, 4 * KATT], F32)  # (b, k)
            for tl in c:
                nc.vector.memset(tl[:], 0.0)
            for tl in mix:
                nc.vector.memset(tl[:], 0.0)
                # boot the bias lane (1.0 at row 76 only) from HB cols 4:8
                nc.vector.tensor_copy(out=tl[64:96, :], in_=sHB[64:96, 4:8])
            nc.vector.memset(kap[:], 0.0)
            nc.vector.memset(h3all[:], 0.0)
            nc.vector.memset(h3all[96:128, :], 1.0)      # MDN bias lane (p96)
            # h init: zeros; attention-bias 1.0 at (p96, blk3 cols) of h0f
            for tl in h:
                nc.vector.memset(tl[:], 0.0)
            nc.vector.memset(h0f[:], 0.0)
            nc.vector.tensor_copy(out=h0f[96:128, 12:16], in_=sHB[96:128, 12:16])

            with tc.tile_pool(name="psum", bufs=1, space="PSUM") as psum, \
                 tc.tile_pool(name="scratch", bufs=2) as scratch:

                z = [psum.tile([128, 64], F32, name=f"z{i}") for i in range(3)]
                attp = psum.tile([1, NB * 30], F32)
                argp = psum.tile([UC, NB * KATT], F32)
                wp = psum.tile([WROWS, NB], F32)
                for tl in z:
                    nc.vector.memset(tl[:], 0.0)

                _sc = {}
                for _t in ["tg", "m1", "m2", "pc2", "zo_", "to", "og", "tcn"]:
                    _sc[_t] = scratch.tile([128, 16], F32, tag=_t, name=_t)
                for _t in ["pc01", "zif_", "tif", "iffg"]:
                    _sc[_t] = scratch.tile([128, 32], F32, tag=_t, name=_t)
                _sc["htl"] = scratch.tile([16, NB], F16, tag="htl", name="htl")
                for _t, _shp in [("E", [1, NB * 30]), ("kap2", [1, NB * KATT]),
                                 ("bk", [1, NB * KATT]), ("A_", [1, NB * KATT]),
                                 ("bk2", [1, NB * KATT]), ("B_", [1, NB * KATT]),
                                 ("C_", [1, NB * KATT]), ("P", [UC, NB * KATT]),
                                 ("phiaug", [OHROWS, NB])]:
                    _sc[_t] = scratch.tile(_shp, F32, tag=_t, name=_t)
                nc.vector.memset(_sc["phiaug"][:], 0.0)
                nc.vector.memset(_sc["phiaug"][64:65, :], 1.0)  # bias source

                def layer_mms(l, sW, movs):
                    for m in range(16):
                        w0, mw = int(SOFF[m]), MW[m]
                        for ki, (mov, kidx) in enumerate(movs):
                            kp = mov.shape[0]
                            nc.tensor.matmul(
                                z[l][0:mw, 4 * m : 4 * m + 4],
                                sW[0:kp, kidx * 1600 + w0 : kidx * 1600 + w0 + mw],
                                mov,
                                start=(ki == 0),
                                stop=(ki == len(movs) - 1),
                            )

                def st(tag):
                    return _sc[tag]

                def cell(l, t):
                    zt = z[l]
                    zg, zo = zt[:, 32:48], zt[:, 48:64]
                    p01 = sPB[:, (3 * l) * 16 : (3 * l) * 16 + 32]
                    p2 = sPB[:, (3 * l + 2) * 16 : (3 * l + 2) * 16 + 16]
                    ct = c[l]            # [128, 32] = [c | c]
                    pc01 = st("pc01")
                    nc.vector.tensor_tensor(out=pc01[:], in0=p01, in1=ct[:],
                                            op=ALU.mult)
                    zif_ = st("zif_")
                    nc.vector.tensor_tensor(out=zif_[:], in0=zt[:, 0:32],
                                            in1=pc01[:], op=ALU.add)
                    tif = st("tif")
                    nc.scalar.activation(out=tif[:], in_=zif_[:], func=AF.Tanh,
                                         scale=0.5)
                    iffg = st("iffg")
                    nc.vector.tensor_scalar(out=iffg[:], in0=tif[:], scalar1=0.5,
                                            scalar2=0.5, op0=ALU.mult, op1=ALU.add)
                    tg = st("tg")
                    nc.scalar.activation(out=tg[:], in_=zg, func=AF.Tanh)
                    m1 = st("m1")
                    nc.vector.tensor_tensor(out=m1[:], in0=iffg[:, 0:16], in1=tg[:],
                                            op=ALU.mult)
                    m2 = st("m2")
                    nc.vector.tensor_tensor(out=m2[:], in0=iffg[:, 16:32],
                                            in1=ct[:, 0:16], op=ALU.mult)
                    nc.vector.tensor_tensor(out=ct[:, 0:16], in0=m1[:], in1=m2[:],
                                            op=ALU.add)
                    nc.vector.tensor_tensor(out=ct[:, 16:32], in0=m1[:], in1=m2[:],
                                            op=ALU.add)
                    pc2 = st("pc2")
                    nc.vector.tensor_tensor(out=pc2[:], in0=p2, in1=ct[:, 0:16],
                                            op=ALU.mult)
                    zo_ = st("zo_")
                    nc.vector.tensor_tensor(out=zo_[:], in0=zo, in1=pc2[:], op=ALU.add)
                    to = st("to")
                    nc.scalar.activation(out=to[:], in_=zo_[:], func=AF.Tanh, scale=0.5)
                    og = st("og")
                    nc.vector.tensor_scalar(out=og[:], in0=to[:], scalar1=0.5,
                                            scalar2=0.5, op0=ALU.mult, op1=ALU.add)
                    tcn = st("tcn")
                    nc.scalar.activation(out=tcn[:], in_=ct[:, 0:16], func=AF.Tanh)
                    # h update: blocks 0-2 to h tile; unit-tail (psum p0:16)
                    # computed at p0:16 then bridged to mix rows 96:112
                    nc.vector.tensor_tensor(out=h[l][:, 0:12], in0=og[:, 0:12],
                                            in1=tcn[:, 0:12], op=ALU.mult)
                    htl = st("htl")
                    nc.vector.tensor_tensor(out=htl[0:16, :],
                                            in0=og[0:16, 12:16],
                                            in1=tcn[0:16, 12:16], op=ALU.mult)
                    nc.vector.tensor_copy(out=mix[l][TAILP:TAILP + 16, :],
                                          in_=htl[0:16, :])
                    if l == 0:
                        # attention reads h0f (fp32): main blocks + blk3
                        # (tail at p0:16, bias 1.0 at p96 kept by 0:96 write)
                        nc.vector.tensor_tensor(out=h0f[:, 0:12],
                                                in0=og[:, 0:12],
                                                in1=tcn[:, 0:12], op=ALU.mult)
                        nc.vector.tensor_tensor(
                            out=h0f[0:96, 12:16],
                            in0=og[0:96, 12:16], in1=tcn[0:96, 12:16],
                            op=ALU.mult)
                    if l == 2:
                        nc.vector.tensor_copy(
                            out=h3all[:, bass.DynSlice(t * 16, 12)],
                            in_=h[2][:, 0:12])
                        nc.vector.tensor_tensor(
                            out=h3all[0:96, bass.DynSlice(t * 16 + 12, 4)],
                            in0=og[0:96, 12:16], in1=tcn[0:96, 12:16],
                            op=ALU.mult)

                def attention(t):
                    for b in range(NB):
                        for k in range(4):
                            nc.tensor.matmul(
                                attp[0:1, 30 * b : 30 * b + 30],
                                h0f[:, 4 * k + b : 4 * k + b + 1],
                                sWATT[:, 30 * k : 30 * k + 30],
                                start=(k == 0), stop=(k == 3),
                            )
                    E = _sc["E"]
                    nc.scalar.activation(out=E[:], in_=attp[:], func=AF.Exp)
                    Ev = E[0:1, :].rearrange("p (b x) -> p b x", b=NB)
                    av = attp[0:1, :].rearrange("p (b x) -> p b x", b=NB)
                    kapv = kap[0:1, :].rearrange("p (b k) -> p b k", b=NB)
                    kap2 = _sc["kap2"]
                    k2v = kap2[0:1, :].rearrange("p (b k) -> p b k", b=NB)
                    nc.vector.tensor_tensor(out=k2v, in0=kapv, in1=Ev[:, :, 20:30],
                                            op=ALU.add)
                    nc.vector.tensor_copy(out=kap[:], in_=kap2[:])
                    bk = _sc["bk"]
                    bkv = bk[0:1, :].rearrange("p (b k) -> p b k", b=NB)
                    nc.vector.tensor_tensor(out=bkv, in0=Ev[:, :, 10:20], in1=k2v,
                                            op=ALU.mult)
                    A_ = _sc["A_"]
                    Av = A_[0:1, :].rearrange("p (b k) -> p b k", b=NB)
                    bk2 = _sc["bk2"]
                    bk2v = bk2[0:1, :].rearrange("p (b k) -> p b k", b=NB)
                    nc.vector.tensor_tensor(out=bk2v, in0=bkv, in1=k2v, op=ALU.mult)
                    nc.vector.tensor_tensor(out=Av, in0=av[:, :, 0:10], in1=bk2v,
                                            op=ALU.subtract)
                    B_ = _sc["B_"]
                    nc.vector.tensor_scalar(out=B_[:], in0=bk[:], scalar1=2.0,
                                            scalar2=None, op0=ALU.mult)
                    C_ = _sc["C_"]
                    Cv = C_[0:1, :].rearrange("p (b k) -> p b k", b=NB)
                    nc.vector.tensor_scalar(out=Cv, in0=Ev[:, :, 10:20], scalar1=-1.0,
                                            scalar2=None, op0=ALU.mult)
                    # arg[u,(b,k)] = A + u*B + u^2*C  via 3 accumulating K=1 matmuls
                    nc.tensor.matmul(argp[:], sV3[0:1, 0:UC], A_[:],
                                     start=True, stop=False)
                    nc.tensor.matmul(argp[:], sV3[0:1, UC:2 * UC], B_[:],
                                     start=False, stop=False)
                    nc.tensor.matmul(argp[:], sV3[0:1, 2 * UC:3 * UC], C_[:],
                                     start=False, stop=True)
                    P = _sc["P"]
                    nc.scalar.activation(out=P[:], in_=argp[:], func=AF.Exp)
                    phiaug = _sc["phiaug"]
                    Pv = P[:, :].rearrange("p (b k) -> p b k", b=NB)
                    nc.vector.tensor_reduce(out=phiaug[0:UC, :], in_=Pv,
                                            axis=mybir.AxisListType.X, op=ALU.add)
                    # stage x_t at p96:99 (feeds layers 2/3 via the wp matmul)
                    nc.vector.tensor_copy(out=phiaug[96:99, :],
                                          in_=sXT[0:3, bass.ts(t, NB)])
                    for b in range(NB):
                        nc.tensor.matmul(
                            wp[:, b : b + 1],
                            sOHB[:, WROWS * b : WROWS * b + WROWS],
                            phiaug[:, b : b + 1],
                            start=True, stop=True,
                        )
                    for l in range(3):
                        nc.vector.tensor_copy(out=mix[l][0:WROWS, :], in_=wp[:])

                with tc.For_i(0, T) as _ts:
                    for t in [_ts]:
                        # refresh x_t for layer 1 (its w slot keeps w_{t-1})
                        nc.vector.tensor_copy(out=mix[0][64:67, :],
                                              in_=sXT[0:3, bass.ts(t, NB)])
                        layer_mms(0, sW1, [(h[0][:, 0:4], 0), (h[0][:, 4:8], 1),
                                           (h[0][:, 8:12], 2), (mix[0][:], 3)])
                        cell(0, t)
                        attention(t)
                        layer_mms(1, sW2, [(h[1][:, 0:4], 0), (h[1][:, 4:8], 1),
                                           (h[1][:, 8:12], 2), (mix[1][:], 3),
                                           (h[0][:, 0:4], 4), (h[0][:, 4:8], 5),
                                           (h[0][:, 8:12], 6), (mix[0][:], 7)])
                        cell(1, t)
                        layer_mms(2, sW3, [(h[2][:, 0:4], 0), (h[2][:, 4:8], 1),
                                           (h[2][:, 8:12], 2), (mix[2][:], 3),
                                           (h[1][:, 0:4], 4), (h[1][:, 4:8], 5),
                                           (h[1][:, 8:12], 6), (mix[1][:], 7)])
                        cell(2, t)

            # ---- MDN head ----
            # Y1 rows: mu @0:40, eos @64, rho @96:116 ; Y2 rows: pi @0:20, s @32:72
            with tc.tile_pool(name="mpsum", bufs=2, space="PSUM") as mpsum, \
                 tc.tile_pool(name="mscr", bufs=2) as mscr, \
                 tc.tile_pool(name="mones", bufs=1) as mones:
                ones20 = mones.tile([NMIX, 1], F32)
                nc.vector.memset(ones20[:], 1.0)
                ones1_20 = mones.tile([1, NMIX], F32)
                nc.vector.memset(ones1_20[:], 1.0)
                h3v = h3all[:, :].rearrange("p (t x) -> p t x", t=T)
                CC = min(400, T * NB)
                TC = CC // NB
                for ch in range((T + TC - 1) // TC):
                    t0 = TC * ch
                    tn = min(TC, T - t0)
                    cc = tn * NB
                    yp1 = mpsum.tile([128, CC], F32, tag="yp1")
                    yp2 = mpsum.tile([72, CC], F32, tag="yp2")
                    for k in range(4):
                        nc.tensor.matmul(
                            yp1[0:128, 0:cc],
                            sWMDN[:, 200 * k : 200 * k + 128],
                            h3v[:, t0 : t0 + tn, 4 * k : 4 * k + 4],
                            start=(k == 0), stop=(k == 3))
                    for k in range(4):
                        nc.tensor.matmul(
                            yp2[0:72, 0:cc],
                            sWMDN[:, 200 * k + 128 : 200 * k + 200],
                            h3v[:, t0 : t0 + tn, 4 * k : 4 * k + 4],
                            start=(k == 0), stop=(k == 3))
                    o1 = OUTS1[:, NB * t0 : NB * t0 + cc]
                    o2 = OUTS2[:, NB * t0 : NB * t0 + cc]
                    # pi softmax (pi lives at yp2[0:20])
                    epi = mscr.tile([NMIX, CC], F32, tag="epi")
                    nc.scalar.activation(out=epi[0:NMIX, 0:cc], in_=yp2[0:NMIX, 0:cc],
                                         func=AF.Exp)
                    sp = mpsum.tile([1, CC], F32, tag="sp")
                    nc.tensor.matmul(sp[0:1, 0:cc], ones20[:], epi[0:NMIX, 0:cc],
                                     start=True, stop=True)
                    rec = mscr.tile([1, CC], F32, tag="rec")
                    nc.vector.reciprocal(out=rec[0:1, 0:cc], in_=sp[0:1, 0:cc])
                    bp = mpsum.tile([NMIX, CC], F32, tag="bp")
                    nc.tensor.matmul(bp[0:NMIX, 0:cc], ones1_20[:], rec[0:1, 0:cc],
                                     start=True, stop=True)
                    nc.vector.tensor_tensor(out=o2[0:20, :], in0=epi[0:NMIX, 0:cc],
                                            in1=bp[0:NMIX, 0:cc], op=ALU.mult)
                    # mu copy (yp1[0:40])
                    nc.vector.tensor_copy(out=o1[0:40, :], in_=yp1[0:40, 0:cc])
                    # rho tanh (yp1[96:116])
                    nc.scalar.activation(out=o1[96:116, :], in_=yp1[96:116, 0:cc],
                                         func=AF.Tanh)
                    # eos sigmoid via tanh (yp1[64:65])
                    teos = mscr.tile([65, CC], F32, tag="teos")
                    nc.scalar.activation(out=teos[64:65, 0:cc], in_=yp1[64:65, 0:cc],
                                         func=AF.Tanh, scale=0.5)
                    nc.vector.tensor_scalar(out=o1[64:65, :], in0=teos[64:65, 0:cc],
                                            scalar1=0.5, scalar2=0.5,
                                            op0=ALU.mult, op1=ALU.add)
                    # s exp (yp2[32:72], split at quadrant boundary)
                    nc.scalar.activation(out=o2[32:64, :], in_=yp2[32:64, 0:cc],
                                         func=AF.Exp)
                    nc.scalar.activation(out=o2[64:72, :], in_=yp2[64:72, 0:cc],
                                         func=AF.Exp)
            nc.gpsimd.dma_start(out=dOUT1[:], in_=OUTS1[:])
            nc.gpsimd.dma_start(out=dOUT2[:], in_=OUTS2[:])

    nc.compile()
    return nc


def _mix_block(Wx, bias, tailW):
    """Weight rows for a mix k-tile: chars at perm rows, x at 64:67,
    bias at 76, unit-tail at 96:112.  Wx rows: [x(3), w(73), ...]."""
    blk = np.zeros((128, 1600), np.float32)
    for ch in range(NCHARS):
        blk[_char_row(ch)] = Wx[3 + ch]
    blk[64:67] = Wx[0:3]
    blk[76] = bias
    if tailW is not None:
        blk[TAILP:TAILP + 16] = tailW
    return blk


def _prep_core(inputs, bsl, T):
    x = np.asarray(inputs['input_strokes'], np.float32)
    chars = np.asarray(inputs['input_chars'])
    lens = np.asarray(inputs['input_char_lens'])

    def W_of(l):
        Wx = np.asarray(inputs['Wx%d' % l], np.float32)
        Wh = np.asarray(inputs['Wh%d' % l], np.float32)
        b = np.asarray(inputs['b%d' % l], np.float32)
        nkt = 4 if l == 0 else 8
        Wfull = np.zeros((128 * nkt, 1600), np.float32)
        Wfull[0:384] = Wh[0:384]
        Wfull[384:512] = _mix_block(Wx, b, Wh[384:400])
        if l > 0:
            # h_prev lives at Wx rows 76:476
            Wfull[512:896] = Wx[76:460]
            Wfull[896 + TAILP : 896 + TAILP + 16] = Wx[460:476]
        out = np.zeros((128, nkt * 1600), np.float32)
        for k in range(nkt):
            out[:, k * 1600 : (k + 1) * 1600] = Wfull[128 * k : 128 * (k + 1)]
        return out

    W1, W2, W3 = W_of(0), W_of(1), W_of(2)

    PB = np.zeros((128, 9 * 16), np.float32)
    for l in range(3):
        p = np.asarray(inputs['p%d' % l], np.float32)
        for j in range(3):
            pbv = np.zeros((128, 16), np.float32)
            for blk in range(3):
                pbv[:, 4 * blk : 4 * blk + 4] = p[j][128 * blk : 128 * blk + 128, None]
            pbv[0:16, 12:16] = p[j][384:400, None]
            PB[:, (3 * l + j) * 16 : (3 * l + j) * 16 + 16] = pbv

    XT = np.zeros((3, T * NB), np.float32)
    xs = x[bsl]
    for b in range(NB):
        XT[:, b::NB] = xs[b].T
    WATT = np.zeros((128, 4 * 30), np.float32)
    wa = np.asarray(inputs['W_att'], np.float32)
    for k in range(3):
        WATT[0:128, 30 * k : 30 * k + 30] = wa[128 * k : 128 * k + 128]
    WATT[0:16, 90:120] = wa[384:400]
    WATT[96, 90:120] = np.asarray(inputs['b_att'], np.float32)
    V3 = np.concatenate([np.ones(UC), np.arange(UC),
                         np.arange(UC) ** 2]).astype(np.float32)[None, :]
    OHB = np.zeros((OHROWS, NB * WROWS), np.float32)
    for b, gb in enumerate(bsl):
        blk = np.zeros((OHROWS, WROWS), np.float32)
        cseq = chars[gb].astype(int)
        n = int(lens[gb])
        for u in range(min(n, UC)):
            blk[u, _char_row(cseq[u])] = 1.0
        blk[64, 76] = 1.0                     # bias lane source
        blk[96:99, 64:67] = np.eye(3, dtype=np.float32)
        OHB[:, WROWS * b : WROWS * b + WROWS] = blk
    # WMDN: per k-tile block [m1(128) | m2(72)]
    wm = np.asarray(inputs['W_mdn'], np.float32)
    bm = np.asarray(inputs['b_mdn'], np.float32)
    wmf = np.zeros((512, 121), np.float32)
    wmf[0:400] = wm
    wmf[3 * 128 + 96] = bm                   # bias via h3all p96 blk3 == 1.0
    m1 = np.zeros((512, 128), np.float32)
    m2 = np.zeros((512, 72), np.float32)
    m1[:, 0:40] = wmf[:, 20:60]             # mu1, mu2
    m1[:, 64:65] = wmf[:, 120:121]          # eos
    m1[:, 96:116] = wmf[:, 100:120]         # rho
    m2[:, 0:20] = wmf[:, 0:20]              # pi
    m2[:, 32:72] = wmf[:, 60:100]           # s1, s2
    WMDN = np.zeros((128, 4 * 200), np.float32)
    for k in range(4):
        WMDN[:, 200 * k : 200 * k + 128] = m1[128 * k : 128 * k + 128]
        WMDN[:, 200 * k + 128 : 200 * k + 200] = m2[128 * k : 128 * k + 128]
    HB = np.zeros((128, 16), np.float32)
    HB[96, 12:16] = 1.0                     # attention-bias lane
    HB[76, 4:8] = 1.0                       # mix bias-boot source
    out = {'W1': W1, 'W2': W2, 'W3': W3, 'PB': PB, 'XT': XT, 'WATT': WATT,
           'V3': V3, 'OHB': OHB, 'WMDN': WMDN, 'HB': HB}
    for k in F16_KEYS:
        out[k] = out[k].astype(np.float16)
    return out


def kernel(**inputs):
    x = np.asarray(inputs['input_strokes'])
    B, T, _ = x.shape
    if T not in _CACHE:
        _CACHE[T] = _build_program(T)
    nc = _CACHE[T]
    in_maps = [_prep_core(inputs, list(range(cr * NB, cr * NB + NB)), T)
               for cr in range(NCORES)]
    res = run_bass_kernel_spmd(nc, in_maps, list(range(NCORES)))
    outs = []
    for cr in range(NCORES):
        O1 = res.results[cr]['OUT1'].reshape(128, T, NB)
        O2 = res.results[cr]['OUT2'].reshape(72, T, NB)
        y = np.empty((NB, T, NOUT), np.float32)
        y[..., 0:20] = O2[0:20].transpose(2, 1, 0)
        y[..., 20:60] = O1[0:40].transpose(2, 1, 0)
        y[..., 60:100] = O2[32:72].transpose(2, 1, 0)
        y[..., 100:120] = O1[96:116].transpose(2, 1, 0)
        y[..., 120:121] = O1[64:65].transpose(2, 1, 0)
        outs.append(y)
    return np.concatenate(outs, 0).astype(np.float32)


# revision 42
# speedup vs baseline: 1.1494x; 1.1494x over previous
import sys
sys.path.insert(0, '/opt/trn_rl_repo')
import numpy as np
import concourse.bass as bass
import concourse.bacc as bacc
import concourse.tile as tile
from concourse import mybir
from concourse.bass_utils import run_bass_kernel_spmd

F32 = mybir.dt.float32
F16 = mybir.dt.float16      # half precision for matmul operands
AF = mybir.ActivationFunctionType
ALU = mybir.AluOpType

U = 400        # LSTM units
KATT = 10     # attention gaussians
NCHARS = 73   # alphabet
NMIX = 20     # GMM components
UC = 50       # char positions
NB = 4        # batch per core
NCORES = 8
NOUT = 6 * NMIX + 1  # 121

# m-tiles: per gate [128,128,128,16] -> 16 m-tiles, psum z [128, 64]
MW = [128, 128, 128, 16] * 4
SOFF = [400 * (m // 4) + 128 * (m % 4) for m in range(16)]   # W col offsets
TAILP = 96          # partition base for unit-tail (384:400) values
WROWS = 77          # mix rows: chars (perm), x @64:67, bias @76
OHROWS = 99         # phiaug/OHB rows: phi @0:50, one @64, x @96:99

_CACHE = {}
F16_KEYS = {'W1', 'W2', 'W3', 'XT', 'WMDN', 'HB'}


def _char_row(c):
    # char c's partition in the mix/wp layout (64:67 reserved for x)
    return c if c < 64 else 67 + (c - 64)


def _build_program(T):
    nc = bacc.Bacc("TRN2", target_bir_lowering=False, debug=False, num_devices=NCORES)

    dW1 = nc.dram_tensor("W1", [128, 4 * 1600], F16, kind="ExternalInput").ap()
    dW2 = nc.dram_tensor("W2", [128, 8 * 1600], F16, kind="ExternalInput").ap()
    dW3 = nc.dram_tensor("W3", [128, 8 * 1600], F16, kind="ExternalInput").ap()
    dPB = nc.dram_tensor("PB", [128, 9 * 16], F32, kind="ExternalInput").ap()
    dXT = nc.dram_tensor("XT", [3, T * NB], F16, kind="ExternalInput").ap()
    dWATT = nc.dram_tensor("WATT", [128, 4 * 30], F32, kind="ExternalInput").ap()
    dV3 = nc.dram_tensor("V3", [1, 3 * UC], F32, kind="ExternalInput").ap()
    dOHB = nc.dram_tensor("OHB", [OHROWS, NB * WROWS], F32, kind="ExternalInput").ap()
    dWMDN = nc.dram_tensor("WMDN", [128, 4 * 200], F16, kind="ExternalInput").ap()
    dHB = nc.dram_tensor("HB", [128, 16], F16, kind="ExternalInput").ap()
    dOUT1 = nc.dram_tensor("OUT1", [128, T * NB], F32, kind="ExternalOutput").ap()
    dOUT2 = nc.dram_tensor("OUT2", [72, T * NB], F32, kind="ExternalOutput").ap()

    with tile.TileContext(nc) as tc:
        with tc.tile_pool(name="statics", bufs=1) as statics, \
             tc.tile_pool(name="states", bufs=1) as states:

            sW1 = statics.tile([128, 4 * 1600], F16)
            sW2 = statics.tile([128, 8 * 1600], F16)
            sW3 = statics.tile([128, 8 * 1600], F16)
            sPB = statics.tile([128, 9 * 16], F32)
            sXT = statics.tile([3, T * NB], F16)
            sWATT = statics.tile([128, 4 * 30], F32)
            sV3 = statics.tile([1, 3 * UC], F32)
            sOHB = statics.tile([OHROWS, NB * WROWS], F32)
            sWMDN = statics.tile([128, 4 * 200], F16)
            sHB = statics.tile([128, 16], F16)
            for dst, src in [(sW1, dW1), (sW2, dW2), (sW3, dW3), (sPB, dPB),
                             (sXT, dXT), (sWATT, dWATT), (sV3, dV3),
                             (sOHB, dOHB), (sWMDN, dWMDN), (sHB, dHB)]:
                nc.gpsimd.dma_start(out=dst[:], in_=src[:])

            h3all = states.tile([128, T * 16], F16)
            OUTS1 = states.tile([128, T * NB], F32)
            OUTS2 = states.tile([72, T * NB], F32)
            h = [states.tile([128, 16], F16, name=f"h{i}") for i in range(3)]
            h0f = states.tile([128, 16], F32, name="h0f")   # fp32 h1 for attention
            # c kept duplicated [c|c] so the i/f gate chains fuse into one op
            c = [states.tile([128, 32], F32, name=f"c{i}") for i in range(3)]
            mix = [states.tile([128, NB], F16, name=f"mix{i}") for i in range(3)]
            kap = states.tile([1, 4 * KATT], F32)  # (b, k)
            for tl in c:
                nc.vector.memset(tl[:], 0.0)
            for tl in mix:
                nc.vector.memset(tl[:], 0.0)
                # boot the bias lane (1.0 at row 76 only) from HB cols 4:8
                nc.vector.tensor_copy(out=tl[64:96, :], in_=sHB[64:96, 4:8])
            nc.vector.memset(kap[:], 0.0)
            nc.vector.memset(h3all[:], 0.0)
            nc.vector.memset(h3all[96:128, :], 1.0)      # MDN bias lane (p96)
            # h init: zeros; attention-bias 1.0 at (p96, blk3 cols) of h0f
            for tl in h:
                nc.vector.memset(tl[:], 0.0)
            nc.vector.memset(h0f[:], 0.0)
            nc.vector.tensor_copy(out=h0f[96:128, 12:16], in_=sHB[96:128, 12:16])

            with tc.tile_pool(name="psum", bufs=1, space="PSUM") as psum, \
                 tc.tile_pool(name="scratch", bufs=2) as scratch:

                z = [psum.tile([128, 64], F32, name=f"z{i}") for i in range(3)]
                attp = psum.tile([1, NB * 30], F32)
                argp = psum.tile([UC, NB * KATT], F32)
                wp = psum.tile([WROWS, NB], F32)
                for tl in z:
                    nc.vector.memset(tl[:], 0.0)

                _sc = {}
                for _t in ["tg", "m1", "m2", "pc2", "zo_", "to", "og", "tcn"]:
                    _sc[_t] = scratch.tile([128, 16], F32, tag=_t, name=_t)
                for _t in ["pc01", "zif_", "tif", "iffg"]:
                    _sc[_t] = scratch.tile([128, 32], F32, tag=_t, name=_t)
                _sc["htl"] = scratch.tile([16, NB], F16, tag="htl", name="htl")
                for _t, _shp in [("E", [1, NB * 30]), ("kap2", [1, NB * KATT]),
                                 ("bk", [1, NB * KATT]), ("A_", [1, NB * KATT]),
                                 ("bk2", [1, NB * KATT]), ("B_", [1, NB * KATT]),
                                 ("C_", [1, NB * KATT]), ("P", [UC, NB * KATT]),
                                 ("phiaug", [OHROWS, NB])]:
                    _sc[_t] = scratch.tile(_shp, F32, tag=_t, name=_t)
                nc.vector.memset(_sc["phiaug"][:], 0.0)
                nc.vector.memset(_sc["phiaug"][64:65, :], 1.0)  # bias source

                def layer_mms(l, sW, movs):
                    for m in range(16):
                        w0, mw = int(SOFF[m]), MW[m]
                        for ki, (mov, kidx) in enumerate(movs):
                            kp = mov.shape[0]
                            nc.tensor.matmul(
                                z[l][0:mw, 4 * m : 4 * m + 4],
                                sW[0:kp, kidx * 1600 + w0 : kidx * 1600 + w0 + mw],
                                mov,
                                start=(ki == 0),
                                stop=(ki == len(movs) - 1),
                            )

                def st(tag):
                    return _sc[tag]

                def cell(l, t):
                    zt = z[l]
                    zg, zo = zt[:, 32:48], zt[:, 48:64]
                    p01 = sPB[:, (3 * l) * 16 : (3 * l) * 16 + 32]
                    p2 = sPB[:, (3 * l + 2) * 16 : (3 * l + 2) * 16 + 16]
                    ct = c[l]            # [128, 32] = [c | c]
                    pc01 = st("pc01")
                    nc.vector.tensor_tensor(out=pc01[:], in0=p01, in1=ct[:],
                                            op=ALU.mult)
                    zif_ = st("zif_")
                    nc.vector.tensor_tensor(out=zif_[:], in0=zt[:, 0:32],
                                            in1=pc01[:], op=ALU.add)
                    tif = st("tif")
                    nc.scalar.activation(out=tif[:], in_=zif_[:], func=AF.Tanh,
                                         scale=0.5)
                    iffg = st("iffg")
                    nc.vector.tensor_scalar(out=iffg[:], in0=tif[:], scalar1=0.5,
                                            scalar2=0.5, op0=ALU.mult, op1=ALU.add)
                    tg = st("tg")
                    nc.scalar.activation(out=tg[:], in_=zg, func=AF.Tanh)
                    m1 = st("m1")
                    nc.vector.tensor_tensor(out=m1[:], in0=iffg[:, 0:16], in1=tg[:],
                                            op=ALU.mult)
                    m2 = st("m2")
                    nc.vector.tensor_tensor(out=m2[:], in0=iffg[:, 16:32],
                                            in1=ct[:, 0:16], op=ALU.mult)
                    nc.vector.tensor_tensor(out=ct[:, 0:16], in0=m1[:], in1=m2[:],
                                            op=ALU.add)
                    nc.vector.tensor_tensor(out=ct[:, 16:32], in0=m1[:], in1=m2[:],
                                            op=ALU.add)
                    pc2 = st("pc2")
                    nc.vector.tensor_tensor(out=pc2[:], in0=p2, in1=ct[:, 0:16],
                                            op=ALU.mult)
                    zo_ = st("zo_")
                    nc.vector.tensor_tensor(out=zo_[:], in0=zo, in1=pc2[:], op=ALU.add)
                    to = st("to")
                    nc.scalar.activation(out=to[:], in_=zo_[:], func=AF.Tanh, scale=0.5)
                    og = st("og")
                    nc.vector.tensor_scalar(out=og[:], in0=to[:], scalar1=0.5,
                                            scalar2=0.5, op0=ALU.mult, op1=ALU.add)
                    tcn = st("tcn")
                    nc.scalar.activation(out=tcn[:], in_=ct[:, 0:16], func=AF.Tanh)
                    # h update: blocks 0-2 to h tile; unit-tail (psum p0:16)
                    # computed at p0:16 then bridged to mix rows 96:112
                    nc.vector.tensor_tensor(out=h[l][:, 0:12], in0=og[:, 0:12],
                                            in1=tcn[:, 0:12], op=ALU.mult)
                    htl = st("htl")
                    nc.vector.tensor_tensor(out=htl[0:16, :],
                                            in0=og[0:16, 12:16],
                                            in1=tcn[0:16, 12:16], op=ALU.mult)
                    nc.vector.tensor_copy(out=mix[l][TAILP:TAILP + 16, :],
                                          in_=htl[0:16, :])
                    if l == 0:
                        # attention reads h0f (fp32): main blocks + blk3
                        # (tail at p0:16, bias 1.0 at p96 kept by 0:96 write)
                        nc.vector.tensor_tensor(out=h0f[:, 0:12],
                                                in0=og[:, 0:12],
                                                in1=tcn[:, 0:12], op=ALU.mult)
                        nc.vector.tensor_tensor(
                            out=h0f[0:96, 12:16],
                            in0=og[0:96, 12:16], in1=tcn[0:96, 12:16],
                            op=ALU.mult)
                    if l == 2:
                        nc.vector.tensor_copy(
                            out=h3all[:, bass.DynSlice(t * 16, 12)],
                            in_=h[2][:, 0:12])
                        nc.vector.tensor_tensor(
                            out=h3all[0:96, bass.DynSlice(t * 16 + 12, 4)],
                            in0=og[0:96, 12:16], in1=tcn[0:96, 12:16],
                            op=ALU.mult)

                def attention(t):
                    for b in range(NB):
                        for k in range(4):
                            nc.tensor.matmul(
                                attp[0:1, 30 * b : 30 * b + 30],
                                h0f[:, 4 * k + b : 4 * k + b + 1],
                                sWATT[:, 30 * k : 30 * k + 30],
                                start=(k == 0), stop=(k == 3),
                            )
                    E = _sc["E"]
                    nc.scalar.activation(out=E[:], in_=attp[:], func=AF.Exp)
                    Ev = E[0:1, :].rearrange("p (b x) -> p b x", b=NB)
                    av = attp[0:1, :].rearrange("p (b x) -> p b x", b=NB)
                    kapv = kap[0:1, :].rearrange("p (b k) -> p b k", b=NB)
                    kap2 = _sc["kap2"]
                    k2v = kap2[0:1, :].rearrange("p (b k) -> p b k", b=NB)
                    nc.vector.tensor_tensor(out=k2v, in0=kapv, in1=Ev[:, :, 20:30],
                                            op=ALU.add)
                    nc.vector.tensor_copy(out=kap[:], in_=kap2[:])
                    bk = _sc["bk"]
                    bkv = bk[0:1, :].rearrange("p (b k) -> p b k", b=NB)
                    nc.vector.tensor_tensor(out=bkv, in0=Ev[:, :, 10:20], in1=k2v,
                                            op=ALU.mult)
                    A_ = _sc["A_"]
                    Av = A_[0:1, :].rearrange("p (b k) -> p b k", b=NB)
                    bk2 = _sc["bk2"]
                    bk2v = bk2[0:1, :].rearrange("p (b k) -> p b k", b=NB)
                    nc.vector.tensor_tensor(out=bk2v, in0=bkv, in1=k2v, op=ALU.mult)
                    nc.vector.tensor_tensor(out=Av, in0=av[:, :, 0:10], in1=bk2v,
                                            op=ALU.subtract)
                    B_ = _sc["B_"]
                    nc.vector.tensor_scalar(out=B_[:], in0=bk[:], scalar1=2.0,
                                            scalar2=None, op0=ALU.mult)
                    C_ = _sc["C_"]
                    Cv = C_[0:1, :].rearrange("p (b k) -> p b k", b=NB)
                    nc.vector.tensor_scalar(out=Cv, in0=Ev[:, :, 10:20], scalar1=-1.0,
                                            scalar2=None, op0=ALU.mult)
                    # arg[u,(b,k)] = A + u*B + u^2*C  via 3 accumulating K=1 matmuls
                    nc.tensor.matmul(argp[:], sV3[0:1, 0:UC], A_[:],
                                     start=True, stop=False)
                    nc.tensor.matmul(argp[:], sV3[0:1, UC:2 * UC], B_[:],
                                     start=False, stop=False)
                    nc.tensor.matmul(argp[:], sV3[0:1, 2 * UC:3 * UC], C_[:],
                                     start=False, stop=True)
                    P = _sc["P"]
                    nc.scalar.activation(out=P[:], in_=argp[:], func=AF.Exp)
                    phiaug = _sc["phiaug"]
                    Pv = P[:, :].rearrange("p (b k) -> p b k", b=NB)
                    nc.vector.tensor_reduce(out=phiaug[0:UC, :], in_=Pv,
                                            axis=mybir.AxisListType.X, op=ALU.add)
                    # stage x_t at p96:99 (feeds layers 2/3 via the wp matmul)
                    nc.vector.tensor_copy(out=phiaug[96:99, :],
                                          in_=sXT[0:3, bass.ts(t, NB)])
                    for b in range(NB):
                        nc.tensor.matmul(
                            wp[:, b : b + 1],
                            sOHB[:, WROWS * b : WROWS * b + WROWS],
                            phiaug[:, b : b + 1],
                            start=True, stop=True,
                        )
                    for l in range(3):
                        nc.vector.tensor_copy(out=mix[l][0:WROWS, :], in_=wp[:])

                def step(t):
                    # refresh x_t for layer 1 (its w slot keeps w_{t-1})
                    nc.vector.tensor_copy(out=mix[0][64:67, :],
                                          in_=sXT[0:3, bass.ts(t, NB)])
                    layer_mms(0, sW1, [(h[0][:, 0:4], 0), (h[0][:, 4:8], 1),
                                       (h[0][:, 8:12], 2), (mix[0][:], 3)])
                    cell(0, t)
                    attention(t)
                    layer_mms(1, sW2, [(h[1][:, 0:4], 0), (h[1][:, 4:8], 1),
                                       (h[1][:, 8:12], 2), (mix[1][:], 3),
                                       (h[0][:, 0:4], 4), (h[0][:, 4:8], 5),
                                       (h[0][:, 8:12], 6), (mix[0][:], 7)])
                    cell(1, t)
                    layer_mms(2, sW3, [(h[2][:, 0:4], 0), (h[2][:, 4:8], 1),
                                       (h[2][:, 8:12], 2), (mix[2][:], 3),
                                       (h[1][:, 0:4], 4), (h[1][:, 4:8], 5),
                                       (h[1][:, 8:12], 6), (mix[1][:], 7)])
                    cell(2, t)

                import os as _os
                UNROLL = int(_os.environ.get("KUNROLL", "4"))
                assert T % UNROLL == 0
                with tc.For_i(0, T // UNROLL) as _ts:
                    for j in range(UNROLL):
                        step(_ts * UNROLL + j)

            # ---- MDN head ----
            # Y1 rows: mu @0:40, eos @64, rho @96:116 ; Y2 rows: pi @0:20, s @32:72
            with tc.tile_pool(name="mpsum", bufs=2, space="PSUM") as mpsum, \
                 tc.tile_pool(name="mscr", bufs=2) as mscr, \
                 tc.tile_pool(name="mones", bufs=1) as mones:
                ones20 = mones.tile([NMIX, 1], F32)
                nc.vector.memset(ones20[:], 1.0)
                ones1_20 = mones.tile([1, NMIX], F32)
                nc.vector.memset(ones1_20[:], 1.0)
                h3v = h3all[:, :].rearrange("p (t x) -> p t x", t=T)
                CC = min(400, T * NB)
                TC = CC // NB
                for ch in range((T + TC - 1) // TC):
                    t0 = TC * ch
                    tn = min(TC, T - t0)
                    cc = tn * NB
                    yp1 = mpsum.tile([128, CC], F32, tag="yp1")
                    yp2 = mpsum.tile([72, CC], F32, tag="yp2")
                    for k in range(4):
                        nc.tensor.matmul(
                            yp1[0:128, 0:cc],
                            sWMDN[:, 200 * k : 200 * k + 128],
                            h3v[:, t0 : t0 + tn, 4 * k : 4 * k + 4],
                            start=(k == 0), stop=(k == 3))
                    for k in range(4):
                        nc.tensor.matmul(
                            yp2[0:72, 0:cc],
                            sWMDN[:, 200 * k + 128 : 200 * k + 200],
                            h3v[:, t0 : t0 + tn, 4 * k : 4 * k + 4],
                            start=(k == 0), stop=(k == 3))
                    o1 = OUTS1[:, NB * t0 : NB * t0 + cc]
                    o2 = OUTS2[:, NB * t0 : NB * t0 + cc]
                    # pi softmax (pi lives at yp2[0:20])
                    epi = mscr.tile([NMIX, CC], F32, tag="epi")
                    nc.scalar.activation(out=epi[0:NMIX, 0:cc], in_=yp2[0:NMIX, 0:cc],
                                         func=AF.Exp)
                    sp = mpsum.tile([1, CC], F32, tag="sp")
                    nc.tensor.matmul(sp[0:1, 0:cc], ones20[:], epi[0:NMIX, 0:cc],
                                     start=True, stop=True)
                    rec = mscr.tile([1, CC], F32, tag="rec")
                    nc.vector.reciprocal(out=rec[0:1, 0:cc], in_=sp[0:1, 0:cc])
                    bp = mpsum.tile([NMIX, CC], F32, tag="bp")
                    nc.tensor.matmul(bp[0:NMIX, 0:cc], ones1_20[:], rec[0:1, 0:cc],
                                     start=True, stop=True)
                    nc.vector.tensor_tensor(out=o2[0:20, :], in0=epi[0:NMIX, 0:cc],
                                            in1=bp[0:NMIX, 0:cc], op=ALU.mult)
                    # mu copy (yp1[0:40])
                    nc.vector.tensor_copy(out=o1[0:40, :], in_=yp1[0:40, 0:cc])
                    # rho tanh (yp1[96:116])
                    nc.scalar.activation(out=o1[96:116, :], in_=yp1[96:116, 0:cc],
                                         func=AF.Tanh)
                    # eos sigmoid via tanh (yp1[64:65])
                    teos = mscr.tile([65, CC], F32, tag="teos")
                    nc.scalar.activation(out=teos[64:65, 0:cc], in_=yp1[64:65, 0:cc],
                                         func=AF.Tanh, scale=0.5)
                    nc.vector.tensor_scalar(out=o1[64:65, :], in0=teos[64:65, 0:cc],
                                            scalar1=0.5, scalar2=0.5,
                                            op0=ALU.mult, op1=ALU.add)
                    # s exp (yp2[32:72], split at quadrant boundary)
                    nc.scalar.activation(out=o2[32:64, :], in_=yp2[32:64, 0:cc],
                                         func=AF.Exp)
                    nc.scalar.activation(out=o2[64:72, :], in_=yp2[64:72, 0:cc],
                                         func=AF.Exp)
            nc.gpsimd.dma_start(out=dOUT1[:], in_=OUTS1[:])
            nc.gpsimd.dma_start(out=dOUT2[:], in_=OUTS2[:])

    nc.compile()
    return nc


def _mix_block(Wx, bias, tailW):
    """Weight rows for a mix k-tile: chars at perm rows, x at 64:67,
    bias at 76, unit-tail at 96:112.  Wx rows: [x(3), w(73), ...]."""
    blk = np.zeros((128, 1600), np.float32)
    for ch in range(NCHARS):
        blk[_char_row(ch)] = Wx[3 + ch]
    blk[64:67] = Wx[0:3]
    blk[76] = bias
    if tailW is not None:
        blk[TAILP:TAILP + 16] = tailW
    return blk


def _prep_core(inputs, bsl, T):
    x = np.asarray(inputs['input_strokes'], np.float32)
    chars = np.asarray(inputs['input_chars'])
    lens = np.asarray(inputs['input_char_lens'])

    def W_of(l):
        Wx = np.asarray(inputs['Wx%d' % l], np.float32)
        Wh = np.asarray(inputs['Wh%d' % l], np.float32)
        b = np.asarray(inputs['b%d' % l], np.float32)
        nkt = 4 if l == 0 else 8
        Wfull = np.zeros((128 * nkt, 1600), np.float32)
        Wfull[0:384] = Wh[0:384]
        Wfull[384:512] = _mix_block(Wx, b, Wh[384:400])
        if l > 0:
            # h_prev lives at Wx rows 76:476
            Wfull[512:896] = Wx[76:460]
            Wfull[896 + TAILP : 896 + TAILP + 16] = Wx[460:476]
        out = np.zeros((128, nkt * 1600), np.float32)
        for k in range(nkt):
            out[:, k * 1600 : (k + 1) * 1600] = Wfull[128 * k : 128 * (k + 1)]
        return out

    W1, W2, W3 = W_of(0), W_of(1), W_of(2)

    PB = np.zeros((128, 9 * 16), np.float32)
    for l in range(3):
        p = np.asarray(inputs['p%d' % l], np.float32)
        for j in range(3):
            pbv = np.zeros((128, 16), np.float32)
            for blk in range(3):
                pbv[:, 4 * blk : 4 * blk + 4] = p[j][128 * blk : 128 * blk + 128, None]
            pbv[0:16, 12:16] = p[j][384:400, None]
            PB[:, (3 * l + j) * 16 : (3 * l + j) * 16 + 16] = pbv

    XT = np.zeros((3, T * NB), np.float32)
    xs = x[bsl]
    for b in range(NB):
        XT[:, b::NB] = xs[b].T
    WATT = np.zeros((128, 4 * 30), np.float32)
    wa = np.asarray(inputs['W_att'], np.float32)
    for k in range(3):
        WATT[0:128, 30 * k : 30 * k + 30] = wa[128 * k : 128 * k + 128]
    WATT[0:16, 90:120] = wa[384:400]
    WATT[96, 90:120] = np.asarray(inputs['b_att'], np.float32)
    V3 = np.concatenate([np.ones(UC), np.arange(UC),
                         np.arange(UC) ** 2]).astype(np.float32)[None, :]
    OHB = np.zeros((OHROWS, NB * WROWS), np.float32)
    for b, gb in enumerate(bsl):
        blk = np.zeros((OHROWS, WROWS), np.float32)
        cseq = chars[gb].astype(int)
        n = int(lens[gb])
        for u in range(min(n, UC)):
            blk[u, _char_row(cseq[u])] = 1.0
        blk[64, 76] = 1.0                     # bias lane source
        blk[96:99, 64:67] = np.eye(3, dtype=np.float32)
        OHB[:, WROWS * b : WROWS * b + WROWS] = blk
    # WMDN: per k-tile block [m1(128) | m2(72)]
    wm = np.asarray(inputs['W_mdn'], np.float32)
    bm = np.asarray(inputs['b_mdn'], np.float32)
    wmf = np.zeros((512, 121), np.float32)
    wmf[0:400] = wm
    wmf[3 * 128 + 96] = bm                   # bias via h3all p96 blk3 == 1.0
    m1 = np.zeros((512, 128), np.float32)
    m2 = np.zeros((512, 72), np.float32)
    m1[:, 0:40] = wmf[:, 20:60]             # mu1, mu2
    m1[:, 64:65] = wmf[:, 120:121]          # eos
    m1[:, 96:116] = wmf[:, 100:120]         # rho
    m2[:, 0:20] = wmf[:, 0:20]              # pi
    m2[:, 32:72] = wmf[:, 60:100]           # s1, s2
    WMDN = np.zeros((128, 4 * 200), np.float32)
    for k in range(4):
        WMDN[:, 200 * k : 200 * k + 128] = m1[128 * k : 128 * k + 128]
        WMDN[:, 200 * k + 128 : 200 * k + 200] = m2[128 * k : 128 * k + 128]
    HB = np.zeros((128, 16), np.float32)
    HB[96, 12:16] = 1.0                     # attention-bias lane
    HB[76, 4:8] = 1.0                       # mix bias-boot source
    out = {'W1': W1, 'W2': W2, 'W3': W3, 'PB': PB, 'XT': XT, 'WATT': WATT,
           'V3': V3, 'OHB': OHB, 'WMDN': WMDN, 'HB': HB}
    for k in F16_KEYS:
        out[k] = out[k].astype(np.float16)
    return out


def kernel(**inputs):
    x = np.asarray(inputs['input_strokes'])
    B, T, _ = x.shape
    if T not in _CACHE:
        _CACHE[T] = _build_program(T)
    nc = _CACHE[T]
    in_maps = [_prep_core(inputs, list(range(cr * NB, cr * NB + NB)), T)
               for cr in range(NCORES)]
    res = run_bass_kernel_spmd(nc, in_maps, list(range(NCORES)))
    outs = []
    for cr in range(NCORES):
        O1 = res.results[cr]['OUT1'].reshape(128, T, NB)
        O2 = res.results[cr]['OUT2'].reshape(72, T, NB)
        y = np.empty((NB, T, NOUT), np.float32)
        y[..., 0:20] = O2[0:20].transpose(2, 1, 0)
        y[..., 20:60] = O1[0:40].transpose(2, 1, 0)
        y[..., 60:100] = O2[32:72].transpose(2, 1, 0)
        y[..., 100:120] = O1[96:116].transpose(2, 1, 0)
        y[..., 120:121] = O1[64:65].transpose(2, 1, 0)
        outs.append(y)
    return np.concatenate(outs, 0).astype(np.float32)


# revision 48
# speedup vs baseline: 1.2057x; 1.0490x over previous
import sys
sys.path.insert(0, '/opt/trn_rl_repo')
import numpy as np
import concourse.bass as bass
import concourse.bacc as bacc
import concourse.tile as tile
from concourse import mybir
from concourse.bass_utils import run_bass_kernel_spmd

F32 = mybir.dt.float32
F16 = mybir.dt.float16      # half precision for matmul operands
AF = mybir.ActivationFunctionType
ALU = mybir.AluOpType

U = 400        # LSTM units
KATT = 10     # attention gaussians
NCHARS = 73   # alphabet
NMIX = 20     # GMM components
UC = 50       # char positions
NB = 4        # batch per core
NCORES = 8
NOUT = 6 * NMIX + 1  # 121

# m-tiles: per gate [128,128,128,16] -> 16 m-tiles, psum z [128, 64]
MW = [128, 128, 128, 16] * 4
SOFF = [400 * (m // 4) + 128 * (m % 4) for m in range(16)]   # W col offsets
TAILP = 96          # partition base for unit-tail (384:400) values
WROWS = 77          # mix rows: chars (perm), x @64:67, bias @76
OHROWS = 99         # phiaug/OHB rows: phi @0:50, one @64, x @96:99

_CACHE = {}
F16_KEYS = {'W1', 'W2', 'W3', 'XT', 'WMDN', 'HB'}


def _char_row(c):
    # char c's partition in the mix/wp layout (64:67 reserved for x)
    return c if c < 64 else 67 + (c - 64)


def _build_program(T):
    nc = bacc.Bacc("TRN2", target_bir_lowering=False, debug=False, num_devices=NCORES)

    dW1 = nc.dram_tensor("W1", [128, 4 * 1600], F16, kind="ExternalInput").ap()
    dW2 = nc.dram_tensor("W2", [128, 8 * 1600], F16, kind="ExternalInput").ap()
    dW3 = nc.dram_tensor("W3", [128, 8 * 1600], F16, kind="ExternalInput").ap()
    dPB = nc.dram_tensor("PB", [128, 9 * 16], F32, kind="ExternalInput").ap()
    dXT = nc.dram_tensor("XT", [3, T * NB], F16, kind="ExternalInput").ap()
    dWATT = nc.dram_tensor("WATT", [128, 4 * 30], F32, kind="ExternalInput").ap()
    dV3 = nc.dram_tensor("V3", [1, 3 * UC], F32, kind="ExternalInput").ap()
    dOHB = nc.dram_tensor("OHB", [OHROWS, NB * WROWS], F32, kind="ExternalInput").ap()
    dWMDN = nc.dram_tensor("WMDN", [128, 4 * 200], F16, kind="ExternalInput").ap()
    dHB = nc.dram_tensor("HB", [128, 16], F16, kind="ExternalInput").ap()
    dOUT1 = nc.dram_tensor("OUT1", [128, T * NB], F32, kind="ExternalOutput").ap()
    dOUT2 = nc.dram_tensor("OUT2", [72, T * NB], F32, kind="ExternalOutput").ap()

    with tile.TileContext(nc) as tc:
        with tc.tile_pool(name="statics", bufs=1) as statics, \
             tc.tile_pool(name="states", bufs=1) as states:

            sW1 = statics.tile([128, 4 * 1600], F16)
            sW2 = statics.tile([128, 8 * 1600], F16)
            sW3 = statics.tile([128, 8 * 1600], F16)
            sPB = statics.tile([128, 9 * 16], F32)
            sXT = statics.tile([3, T * NB], F16)
            sWATT = statics.tile([128, 4 * 30], F32)
            sV3 = statics.tile([1, 3 * UC], F32)
            sOHB = statics.tile([OHROWS, NB * WROWS], F32)
            sWMDN = statics.tile([128, 4 * 200], F16)
            sHB = statics.tile([128, 16], F16)
            for dst, src in [(sW1, dW1), (sW2, dW2), (sW3, dW3), (sPB, dPB),
                             (sXT, dXT), (sWATT, dWATT), (sV3, dV3),
                             (sOHB, dOHB), (sWMDN, dWMDN), (sHB, dHB)]:
                nc.gpsimd.dma_start(out=dst[:], in_=src[:])

            h3all = states.tile([128, T * 16], F16)
            OUTS1 = states.tile([128, T * NB], F32)
            OUTS2 = states.tile([72, T * NB], F32)
            h = [states.tile([128, 16], F16, name=f"h{i}") for i in range(3)]
            h0f = states.tile([128, 16], F32, name="h0f")   # fp32 h1 for attention
            # c kept duplicated [c|c] so the i/f gate chains fuse into one op
            c = [states.tile([128, 32], F32, name=f"c{i}") for i in range(3)]
            mix = [states.tile([128, NB], F16, name=f"mix{i}") for i in range(3)]
            kap = states.tile([1, 4 * KATT], F32)  # (b, k)
            for tl in c:
                nc.vector.memset(tl[:], 0.0)
            for tl in mix:
                nc.vector.memset(tl[:], 0.0)
                # boot the bias lane (1.0 at row 76 only) from HB cols 4:8
                nc.vector.tensor_copy(out=tl[64:96, :], in_=sHB[64:96, 4:8])
            nc.vector.memset(kap[:], 0.0)
            nc.vector.memset(h3all[:], 0.0)
            nc.vector.memset(h3all[96:128, :], 1.0)      # MDN bias lane (p96)
            # h init: zeros; attention-bias 1.0 at (p96, blk3 cols) of h0f
            for tl in h:
                nc.vector.memset(tl[:], 0.0)
            nc.vector.memset(h0f[:], 0.0)
            nc.vector.tensor_copy(out=h0f[96:128, 12:16], in_=sHB[96:128, 12:16])

            with tc.tile_pool(name="psum", bufs=1, space="PSUM") as psum, \
                 tc.tile_pool(name="scratch", bufs=2) as scratch:

                # full-bank tiles (512 f32, cols 0:64 used) so attention's
                # psum groups never share a bank with a split-open z group
                z = [psum.tile([128, 512], F32, name=f"z{i}") for i in range(3)]
                attp = psum.tile([1, NB * 30], F32)
                argp = psum.tile([UC, NB * KATT], F32)
                wp = psum.tile([WROWS, NB], F32)
                for tl in z:
                    nc.vector.memset(tl[:], 0.0)

                _sc = {}
                for _t in ["tg", "m1", "m2", "pc2", "zo_", "to", "og", "tcn"]:
                    _sc[_t] = scratch.tile([128, 16], F32, tag=_t, name=_t)
                for _t in ["pc01", "zif_", "tif", "iffg"]:
                    _sc[_t] = scratch.tile([128, 32], F32, tag=_t, name=_t)
                _sc["htl"] = scratch.tile([16, NB], F16, tag="htl", name="htl")
                for _t, _shp in [("E", [1, NB * 30]), ("kap2", [1, NB * KATT]),
                                 ("bk", [1, NB * KATT]), ("A_", [1, NB * KATT]),
                                 ("bk2", [1, NB * KATT]), ("B_", [1, NB * KATT]),
                                 ("C_", [1, NB * KATT]), ("P", [UC, NB * KATT]),
                                 ("phiaug", [OHROWS, NB])]:
                    _sc[_t] = scratch.tile(_shp, F32, tag=_t, name=_t)
                nc.vector.memset(_sc["phiaug"][:], 0.0)
                nc.vector.memset(_sc["phiaug"][64:65, :], 1.0)  # bias source

                def layer_mms(l, sW, movs, lo=0, hi=None):
                    nk = len(movs)
                    if hi is None:
                        hi = nk
                    for m in range(16):
                        w0, mw = int(SOFF[m]), MW[m]
                        for ki in range(lo, hi):
                            mov, kidx = movs[ki]
                            kp = mov.shape[0]
                            nc.tensor.matmul(
                                z[l][0:mw, 4 * m : 4 * m + 4],
                                sW[0:kp, kidx * 1600 + w0 : kidx * 1600 + w0 + mw],
                                mov,
                                start=(ki == 0),
                                stop=(ki == nk - 1),
                            )

                def st(tag):
                    return _sc[tag]

                def cell(l, t):
                    zt = z[l]
                    zg, zo = zt[:, 32:48], zt[:, 48:64]
                    p01 = sPB[:, (3 * l) * 16 : (3 * l) * 16 + 32]
                    p2 = sPB[:, (3 * l + 2) * 16 : (3 * l + 2) * 16 + 16]
                    ct = c[l]            # [128, 32] = [c | c]
                    pc01 = st("pc01")
                    nc.vector.tensor_tensor(out=pc01[:], in0=p01, in1=ct[:],
                                            op=ALU.mult)
                    zif_ = st("zif_")
                    nc.vector.tensor_tensor(out=zif_[:], in0=zt[:, 0:32],
                                            in1=pc01[:], op=ALU.add)
                    tif = st("tif")
                    nc.scalar.activation(out=tif[:], in_=zif_[:], func=AF.Tanh,
                                         scale=0.5)
                    iffg = st("iffg")
                    nc.vector.tensor_scalar(out=iffg[:], in0=tif[:], scalar1=0.5,
                                            scalar2=0.5, op0=ALU.mult, op1=ALU.add)
                    tg = st("tg")
                    nc.scalar.activation(out=tg[:], in_=zg, func=AF.Tanh)
                    m1 = st("m1")
                    nc.vector.tensor_tensor(out=m1[:], in0=iffg[:, 0:16], in1=tg[:],
                                            op=ALU.mult)
                    m2 = st("m2")
                    nc.vector.tensor_tensor(out=m2[:], in0=iffg[:, 16:32],
                                            in1=ct[:, 0:16], op=ALU.mult)
                    nc.vector.tensor_tensor(out=ct[:, 0:16], in0=m1[:], in1=m2[:],
                                            op=ALU.add)
                    nc.vector.tensor_tensor(out=ct[:, 16:32], in0=m1[:], in1=m2[:],
                                            op=ALU.add)
                    pc2 = st("pc2")
                    nc.vector.tensor_tensor(out=pc2[:], in0=p2, in1=ct[:, 0:16],
                                            op=ALU.mult)
                    zo_ = st("zo_")
                    nc.vector.tensor_tensor(out=zo_[:], in0=zo, in1=pc2[:], op=ALU.add)
                    to = st("to")
                    nc.scalar.activation(out=to[:], in_=zo_[:], func=AF.Tanh, scale=0.5)
                    og = st("og")
                    nc.vector.tensor_scalar(out=og[:], in0=to[:], scalar1=0.5,
                                            scalar2=0.5, op0=ALU.mult, op1=ALU.add)
                    tcn = st("tcn")
                    nc.scalar.activation(out=tcn[:], in_=ct[:, 0:16], func=AF.Tanh)
                    # h update: blocks 0-2 to h tile; unit-tail (psum p0:16)
                    # computed at p0:16 then bridged to mix rows 96:112
                    nc.vector.tensor_tensor(out=h[l][:, 0:12], in0=og[:, 0:12],
                                            in1=tcn[:, 0:12], op=ALU.mult)
                    htl = st("htl")
                    nc.vector.tensor_tensor(out=htl[0:16, :],
                                            in0=og[0:16, 12:16],
                                            in1=tcn[0:16, 12:16], op=ALU.mult)
                    nc.vector.tensor_copy(out=mix[l][TAILP:TAILP + 16, :],
                                          in_=htl[0:16, :])
                    if l == 0:
                        # attention reads h0f (fp32): main blocks + blk3
                        # (tail at p0:16, bias 1.0 at p96 kept by 0:96 write)
                        nc.vector.tensor_tensor(out=h0f[:, 0:12],
                                                in0=og[:, 0:12],
                                                in1=tcn[:, 0:12], op=ALU.mult)
                        nc.vector.tensor_tensor(
                            out=h0f[0:96, 12:16],
                            in0=og[0:96, 12:16], in1=tcn[0:96, 12:16],
                            op=ALU.mult)
                    if l == 2:
                        nc.vector.tensor_copy(
                            out=h3all[:, bass.DynSlice(t * 16, 12)],
                            in_=h[2][:, 0:12])
                        nc.vector.tensor_tensor(
                            out=h3all[0:96, bass.DynSlice(t * 16 + 12, 4)],
                            in0=og[0:96, 12:16], in1=tcn[0:96, 12:16],
                            op=ALU.mult)

                def attention(t):
                    for b in range(NB):
                        for k in range(4):
                            nc.tensor.matmul(
                                attp[0:1, 30 * b : 30 * b + 30],
                                h0f[:, 4 * k + b : 4 * k + b + 1],
                                sWATT[:, 30 * k : 30 * k + 30],
                                start=(k == 0), stop=(k == 3),
                            )
                    E = _sc["E"]
                    nc.scalar.activation(out=E[:], in_=attp[:], func=AF.Exp)
                    Ev = E[0:1, :].rearrange("p (b x) -> p b x", b=NB)
                    av = attp[0:1, :].rearrange("p (b x) -> p b x", b=NB)
                    kapv = kap[0:1, :].rearrange("p (b k) -> p b k", b=NB)
                    kap2 = _sc["kap2"]
                    k2v = kap2[0:1, :].rearrange("p (b k) -> p b k", b=NB)
                    nc.vector.tensor_tensor(out=k2v, in0=kapv, in1=Ev[:, :, 20:30],
                                            op=ALU.add)
                    nc.vector.tensor_copy(out=kap[:], in_=kap2[:])
                    bk = _sc["bk"]
                    bkv = bk[0:1, :].rearrange("p (b k) -> p b k", b=NB)
                    nc.vector.tensor_tensor(out=bkv, in0=Ev[:, :, 10:20], in1=k2v,
                                            op=ALU.mult)
                    A_ = _sc["A_"]
                    Av = A_[0:1, :].rearrange("p (b k) -> p b k", b=NB)
                    bk2 = _sc["bk2"]
                    bk2v = bk2[0:1, :].rearrange("p (b k) -> p b k", b=NB)
                    nc.vector.tensor_tensor(out=bk2v, in0=bkv, in1=k2v, op=ALU.mult)
                    nc.vector.tensor_tensor(out=Av, in0=av[:, :, 0:10], in1=bk2v,
                                            op=ALU.subtract)
                    B_ = _sc["B_"]
                    nc.vector.tensor_scalar(out=B_[:], in0=bk[:], scalar1=2.0,
                                            scalar2=None, op0=ALU.mult)
                    C_ = _sc["C_"]
                    Cv = C_[0:1, :].rearrange("p (b k) -> p b k", b=NB)
                    nc.vector.tensor_scalar(out=Cv, in0=Ev[:, :, 10:20], scalar1=-1.0,
                                            scalar2=None, op0=ALU.mult)
                    # arg[u,(b,k)] = A + u*B + u^2*C  via 3 accumulating K=1 matmuls
                    nc.tensor.matmul(argp[:], sV3[0:1, 0:UC], A_[:],
                                     start=True, stop=False)
                    nc.tensor.matmul(argp[:], sV3[0:1, UC:2 * UC], B_[:],
                                     start=False, stop=False)
                    nc.tensor.matmul(argp[:], sV3[0:1, 2 * UC:3 * UC], C_[:],
                                     start=False, stop=True)
                    P = _sc["P"]
                    nc.scalar.activation(out=P[:], in_=argp[:], func=AF.Exp)
                    phiaug = _sc["phiaug"]
                    Pv = P[:, :].rearrange("p (b k) -> p b k", b=NB)
                    nc.vector.tensor_reduce(out=phiaug[0:UC, :], in_=Pv,
                                            axis=mybir.AxisListType.X, op=ALU.add)
                    # stage x_t at p96:99 (feeds layers 2/3 via the wp matmul)
                    nc.vector.tensor_copy(out=phiaug[96:99, :],
                                          in_=sXT[0:3, bass.ts(t, NB)])
                    for b in range(NB):
                        nc.tensor.matmul(
                            wp[:, b : b + 1],
                            sOHB[:, WROWS * b : WROWS * b + WROWS],
                            phiaug[:, b : b + 1],
                            start=True, stop=True,
                        )
                    for l in range(3):
                        nc.vector.tensor_copy(out=mix[l][0:WROWS, :], in_=wp[:])

                def step(t):
                    movs1 = [(h[0][:, 0:4], 0), (h[0][:, 4:8], 1),
                             (h[0][:, 8:12], 2), (mix[0][:], 3)]
                    movs2 = [(h[1][:, 0:4], 0), (h[1][:, 4:8], 1),
                             (h[1][:, 8:12], 2), (mix[1][:], 3),
                             (h[0][:, 0:4], 4), (h[0][:, 4:8], 5),
                             (h[0][:, 8:12], 6), (mix[0][:], 7)]
                    movs3 = [(h[2][:, 0:4], 0), (h[2][:, 4:8], 1),
                             (h[2][:, 8:12], 2), (mix[2][:], 3),
                             (h[1][:, 0:4], 4), (h[1][:, 4:8], 5),
                             (h[1][:, 8:12], 6), (mix[1][:], 7)]
                    # refresh x_t for layer 1 (its w slot keeps w_{t-1})
                    nc.vector.tensor_copy(out=mix[0][64:67, :],
                                          in_=sXT[0:3, bass.ts(t, NB)])
                    layer_mms(0, sW1, movs1)
                    cell(0, t)
                    attention(t)
                    layer_mms(1, sW2, movs2)
                    cell(1, t)
                    layer_mms(2, sW3, movs3)
                    cell(2, t)

                import os as _os
                UNROLL = int(_os.environ.get("KUNROLL", "8"))
                assert T % UNROLL == 0
                with tc.For_i(0, T // UNROLL) as _ts:
                    for j in range(UNROLL):
                        step(_ts * UNROLL + j)

            # ---- MDN head ----
            # Y1 rows: mu @0:40, eos @64, rho @96:116 ; Y2 rows: pi @0:20, s @32:72
            with tc.tile_pool(name="mpsum", bufs=2, space="PSUM") as mpsum, \
                 tc.tile_pool(name="mscr", bufs=2) as mscr, \
                 tc.tile_pool(name="mones", bufs=1) as mones:
                ones20 = mones.tile([NMIX, 1], F32)
                nc.vector.memset(ones20[:], 1.0)
                ones1_20 = mones.tile([1, NMIX], F32)
                nc.vector.memset(ones1_20[:], 1.0)
                h3v = h3all[:, :].rearrange("p (t x) -> p t x", t=T)
                CC = min(400, T * NB)
                TC = CC // NB
                for ch in range((T + TC - 1) // TC):
                    t0 = TC * ch
                    tn = min(TC, T - t0)
                    cc = tn * NB
                    yp1 = mpsum.tile([128, CC], F32, tag="yp1")
                    yp2 = mpsum.tile([72, CC], F32, tag="yp2")
                    for k in range(4):
                        nc.tensor.matmul(
                            yp1[0:128, 0:cc],
                            sWMDN[:, 200 * k : 200 * k + 128],
                            h3v[:, t0 : t0 + tn, 4 * k : 4 * k + 4],
                            start=(k == 0), stop=(k == 3))
                    for k in range(4):
                        nc.tensor.matmul(
                            yp2[0:72, 0:cc],
                            sWMDN[:, 200 * k + 128 : 200 * k + 200],
                            h3v[:, t0 : t0 + tn, 4 * k : 4 * k + 4],
                            start=(k == 0), stop=(k == 3))
                    o1 = OUTS1[:, NB * t0 : NB * t0 + cc]
                    o2 = OUTS2[:, NB * t0 : NB * t0 + cc]
                    # pi softmax (pi lives at yp2[0:20])
                    epi = mscr.tile([NMIX, CC], F32, tag="epi")
                    nc.scalar.activation(out=epi[0:NMIX, 0:cc], in_=yp2[0:NMIX, 0:cc],
                                         func=AF.Exp)
                    sp = mpsum.tile([1, CC], F32, tag="sp")
                    nc.tensor.matmul(sp[0:1, 0:cc], ones20[:], epi[0:NMIX, 0:cc],
                                     start=True, stop=True)
                    rec = mscr.tile([1, CC], F32, tag="rec")
                    nc.vector.reciprocal(out=rec[0:1, 0:cc], in_=sp[0:1, 0:cc])
                    bp = mpsum.tile([NMIX, CC], F32, tag="bp")
                    nc.tensor.matmul(bp[0:NMIX, 0:cc], ones1_20[:], rec[0:1, 0:cc],
                                     start=True, stop=True)
                    nc.vector.tensor_tensor(out=o2[0:20, :], in0=epi[0:NMIX, 0:cc],
                                            in1=bp[0:NMIX, 0:cc], op=ALU.mult)
                    # mu copy (yp1[0:40])
                    nc.vector.tensor_copy(out=o1[0:40, :], in_=yp1[0:40, 0:cc])
                    # rho tanh (yp1[96:116])
                    nc.scalar.activation(out=o1[96:116, :], in_=yp1[96:116, 0:cc],
                                         func=AF.Tanh)
                    # eos sigmoid via tanh (yp1[64:65])
                    teos = mscr.tile([65, CC], F32, tag="teos")
                    nc.scalar.activation(out=teos[64:65, 0:cc], in_=yp1[64:65, 0:cc],
                                         func=AF.Tanh, scale=0.5)
                    nc.vector.tensor_scalar(out=o1[64:65, :], in0=teos[64:65, 0:cc],
                                            scalar1=0.5, scalar2=0.5,
                                            op0=ALU.mult, op1=ALU.add)
                    # s exp (yp2[32:72], split at quadrant boundary)
                    nc.scalar.activation(out=o2[32:64, :], in_=yp2[32:64, 0:cc],
                                         func=AF.Exp)
                    nc.scalar.activation(out=o2[64:72, :], in_=yp2[64:72, 0:cc],
                                         func=AF.Exp)
            nc.gpsimd.dma_start(out=dOUT1[:], in_=OUTS1[:])
            nc.gpsimd.dma_start(out=dOUT2[:], in_=OUTS2[:])

    nc.compile()
    return nc


def _mix_block(Wx, bias, tailW):
    """Weight rows for a mix k-tile: chars at perm rows, x at 64:67,
    bias at 76, unit-tail at 96:112.  Wx rows: [x(3), w(73), ...]."""
    blk = np.zeros((128, 1600), np.float32)
    for ch in range(NCHARS):
        blk[_char_row(ch)] = Wx[3 + ch]
    blk[64:67] = Wx[0:3]
    blk[76] = bias
    if tailW is not None:
        blk[TAILP:TAILP + 16] = tailW
    return blk


def _prep_core(inputs, bsl, T):
    x = np.asarray(inputs['input_strokes'], np.float32)
    chars = np.asarray(inputs['input_chars'])
    lens = np.asarray(inputs['input_char_lens'])

    def W_of(l):
        Wx = np.asarray(inputs['Wx%d' % l], np.float32)
        Wh = np.asarray(inputs['Wh%d' % l], np.float32)
        b = np.asarray(inputs['b%d' % l], np.float32)
        nkt = 4 if l == 0 else 8
        Wfull = np.zeros((128 * nkt, 1600), np.float32)
        Wfull[0:384] = Wh[0:384]
        Wfull[384:512] = _mix_block(Wx, b, Wh[384:400])
        if l > 0:
            # h_prev lives at Wx rows 76:476
            Wfull[512:896] = Wx[76:460]
            Wfull[896 + TAILP : 896 + TAILP + 16] = Wx[460:476]
        out = np.zeros((128, nkt * 1600), np.float32)
        for k in range(nkt):
            out[:, k * 1600 : (k + 1) * 1600] = Wfull[128 * k : 128 * (k + 1)]
        return out

    W1, W2, W3 = W_of(0), W_of(1), W_of(2)

    PB = np.zeros((128, 9 * 16), np.float32)
    for l in range(3):
        p = np.asarray(inputs['p%d' % l], np.float32)
        for j in range(3):
            pbv = np.zeros((128, 16), np.float32)
            for blk in range(3):
                pbv[:, 4 * blk : 4 * blk + 4] = p[j][128 * blk : 128 * blk + 128, None]
            pbv[0:16, 12:16] = p[j][384:400, None]
            PB[:, (3 * l + j) * 16 : (3 * l + j) * 16 + 16] = pbv

    XT = np.zeros((3, T * NB), np.float32)
    xs = x[bsl]
    for b in range(NB):
        XT[:, b::NB] = xs[b].T
    WATT = np.zeros((128, 4 * 30), np.float32)
    wa = np.asarray(inputs['W_att'], np.float32)
    for k in range(3):
        WATT[0:128, 30 * k : 30 * k + 30] = wa[128 * k : 128 * k + 128]
    WATT[0:16, 90:120] = wa[384:400]
    WATT[96, 90:120] = np.asarray(inputs['b_att'], np.float32)
    V3 = np.concatenate([np.ones(UC), np.arange(UC),
                         np.arange(UC) ** 2]).astype(np.float32)[None, :]
    OHB = np.zeros((OHROWS, NB * WROWS), np.float32)
    for b, gb in enumerate(bsl):
        blk = np.zeros((OHROWS, WROWS), np.float32)
        cseq = chars[gb].astype(int)
        n = int(lens[gb])
        for u in range(min(n, UC)):
            blk[u, _char_row(cseq[u])] = 1.0
        blk[64, 76] = 1.0                     # bias lane source
        blk[96:99, 64:67] = np.eye(3, dtype=np.float32)
        OHB[:, WROWS * b : WROWS * b + WROWS] = blk
    # WMDN: per k-tile block [m1(128) | m2(72)]
    wm = np.asarray(inputs['W_mdn'], np.float32)
    bm = np.asarray(inputs['b_mdn'], np.float32)
    wmf = np.zeros((512, 121), np.float32)
    wmf[0:400] = wm
    wmf[3 * 128 + 96] = bm                   # bias via h3all p96 blk3 == 1.0
    m1 = np.zeros((512, 128), np.float32)
    m2 = np.zeros((512, 72), np.float32)
    m1[:, 0:40] = wmf[:, 20:60]             # mu1, mu2
    m1[:, 64:65] = wmf[:, 120:121]          # eos
    m1[:, 96:116] = wmf[:, 100:120]         # rho
    m2[:, 0:20] = wmf[:, 0:20]              # pi
    m2[:, 32:72] = wmf[:, 60:100]           # s1, s2
    WMDN = np.zeros((128, 4 * 200), np.float32)
    for k in range(4):
        WMDN[:, 200 * k : 200 * k + 128] = m1[128 * k : 128 * k + 128]
        WMDN[:, 200 * k + 128 : 200 * k + 200] = m2[128 * k : 128 * k + 128]
    HB = np.zeros((128, 16), np.float32)
    HB[96, 12:16] = 1.0                     # attention-bias lane
    HB[76, 4:8] = 1.0                       # mix bias-boot source
    out = {'W1': W1, 'W2': W2, 'W3': W3, 'PB': PB, 'XT': XT, 'WATT': WATT,
           'V3': V3, 'OHB': OHB, 'WMDN': WMDN, 'HB': HB}
    for k in F16_KEYS:
        out[k] = out[k].astype(np.float16)
    return out


def kernel(**inputs):
    x = np.asarray(inputs['input_strokes'])
    B, T, _ = x.shape
    if T not in _CACHE:
        _CACHE[T] = _build_program(T)
    nc = _CACHE[T]
    in_maps = [_prep_core(inputs, list(range(cr * NB, cr * NB + NB)), T)
               for cr in range(NCORES)]
    res = run_bass_kernel_spmd(nc, in_maps, list(range(NCORES)))
    outs = []
    for cr in range(NCORES):
        O1 = res.results[cr]['OUT1'].reshape(128, T, NB)
        O2 = res.results[cr]['OUT2'].reshape(72, T, NB)
        y = np.empty((NB, T, NOUT), np.float32)
        y[..., 0:20] = O2[0:20].transpose(2, 1, 0)
        y[..., 20:60] = O1[0:40].transpose(2, 1, 0)
        y[..., 60:100] = O2[32:72].transpose(2, 1, 0)
        y[..., 100:120] = O1[96:116].transpose(2, 1, 0)
        y[..., 120:121] = O1[64:65].transpose(2, 1, 0)
        outs.append(y)
    return np.concatenate(outs, 0).astype(np.float32)
